# revision 68
# baseline (speedup 1.0000x reference)
"""Multi-head causal attention (B=8, S=2048, E=512, H=8, D=64) on 8 trn2 cores.

Strategy: pure data parallelism over the batch dimension (B == n_cores == 8).
Each NeuronCore computes the full attention for one batch element; no
collectives are needed. All matmul operands are bf16 (stationary loads are
2 bytes/element and use standalone LDWEIGHTS instructions that the PE's
64-deep reorder window pulls ahead of in-flight MATMULs; the fp32r
self-loading path serializes ~270 ns of weight load into every matmul).
PSUM accumulation stays fp32. Per core:

  1. Load x [S,E], cast bf16 (DVE), transpose on PE -> xT [E,S] bf16.
  2. QKV projections:
       qT/kT: per 2-head group g, lhsT = Wq[e-block, (2h,d)=128] -> qT[g] [128,S]
       v:      per s-block, lhsT = xT block, rhs = Wv (4 heads at a time) ->
       v stored interleaved as vt[p=sk, h, j, 0:64] with vt[..., 64] = 1.0 so
       that the AV matmul's stationary operand [128, 65] also produces the
       softmax denominator (row 64 of the output accumulator). The v halves
       and the attention of stripe 0 are interleaved with the qT/kT groups so
       ScalarE's exp stream (the second-longest engine) starts early.
  3. Per sq-stripe (1024 cols) and head: scoresT[sk,sq] = kT_j^T @ qT
       (K=64), causal: only sq >= 128*j is computed; the diagonal 128x128
       block is masked by accumulating a bf16 (-1e30 strictly-lower) mask
       matmul into PSUM on the PE; exp via ScalarE (no max subtraction
       needed: |scores/8| <~ 2) -> bf16 att weights, then
       outT_aug[65,sq] += vt_j^T @ attT_j. Emission is software-pipelined two
       deep (scores_j, AV_{j-2}, exp_j) AND across heads: each head's final
       two AV groups + PSUM->SBUF staging are deferred into the next head's
       emission (after its first scores block), so the PE never head-of-line
       blocks on ScalarE's exp at head boundaries.
  4. Normalize, batched per stripe: each head's unnormalized output and
       denominator row l are staged to SBUF (l rows gathered onto 8
       partitions of one tile via SBUF->SBUF DMA); ONE DVE reciprocal per
       stripe covers all 8 heads (the op costs ~6.5 ns/free-elem regardless
       of partition count); r is rounded to bf16 and broadcast across 64
       partitions with a K=8 one-hot-row-selector matmul on the PE
       (sel_h^T @ r -> PSUM); attoutT[hd, s] = staged * r on DVE (PSUM
       read). The stripe-t norm chain is emitted after stripe t+1's first
       head so its serial DVE/DMA latency hides under PE work.
  5. Output projection out[s,e] = attoutT^T @ Wp + bp -> HBM, interleaved
       with the next stripe's attention.

A single PSUM pool with two tags (4+4 banks) is used for the whole kernel.
Post-scheduling, excess semaphore waits are hoisted onto same-engine NoOps
(several ISA structs accept only one wait slot; walrus rejects multi-wait
instructions)."""

import numpy as np
from contextlib import ExitStack

import concourse.bass as bass
import concourse.mybir as mybir
from concourse.tile import TileContext
from concourse.masks import make_identity
from concourse import bass_utils

F32 = mybir.dt.float32
BF16 = mybir.dt.bfloat16
B, S_FULL, E, H, D = 8, 2048, 512, 8, 64
P = 128
G = H // 2      # 2-head groups
EB = E // P     # e blocks
NEG = -1.0e30


def build_attention_nc(S=S_FULL):
    SB = S // P                 # s blocks
    SW = min(1024, S)           # stripe width (sq columns)
    NS = S // SW                # number of stripes
    QW = min(1024, S)           # qkT psum chunk width
    nc = bass.Bass(trn_type="TRN2")

    x_d = nc.dram_tensor("x", [S, E], F32, kind="ExternalInput").ap()
    wq_d = nc.dram_tensor("Wq", [H, E, D], F32, kind="ExternalInput").ap()
    wk_d = nc.dram_tensor("Wk", [H, E, D], F32, kind="ExternalInput").ap()
    wv_d = nc.dram_tensor("Wv", [H, E, D], F32, kind="ExternalInput").ap()
    wp_d = nc.dram_tensor("Wp", [E, E], F32, kind="ExternalInput").ap()
    bp_d = nc.dram_tensor("bp", [E], F32, kind="ExternalInput").ap()
    out_d = nc.dram_tensor("out", [S, E], F32, kind="ExternalOutput").ap()

    with TileContext(nc) as tc, ExitStack() as top:
        const = top.enter_context(tc.tile_pool(name="const", bufs=1))
        # bf16 identity: mask matmuls + bf16 transposes run at 1 cyc/row
        identb = const.tile([P, P], BF16, tag="identb")
        make_identity(nc, identb)
        # umask[sk, sq] = NEG where sq < sk else 0  (strictly-lower triangle)
        umask = const.tile([P, P], BF16, tag="umask")
        nc.gpsimd.memset(umask, 0.0)
        nc.gpsimd.affine_select(
            out=umask, in_=umask, compare_op=mybir.AluOpType.is_ge,
            fill=NEG, base=0, pattern=[[1, P]], channel_multiplier=-1,
        )
        bpb = const.tile([P, E], F32, tag="bpb")
        wp_sb = const.tile([P, G, E], BF16, tag="wp")
        # sel[h]: [H, D] one-hot row selector (row h all-ones) — K=8 matmul
        # lhsT that picks partition h of the rhs and broadcasts it 64-wide
        sel = []
        for h in range(H):
            s = const.tile([H, D], BF16, tag=f"sel{h}")
            nc.gpsimd.memset(s, 1.0)
            nc.gpsimd.affine_select(
                out=s, in_=s, compare_op=mybir.AluOpType.is_equal,
                fill=0.0, base=-h, pattern=[[0, D]], channel_multiplier=1,
            )
            sel.append(s)

        out_pool = top.enter_context(tc.tile_pool(name="outsb", bufs=2))
        qkv = top.enter_context(tc.tile_pool(name="qkv", bufs=1))
        qT = [qkv.tile([P, S], BF16, tag=f"qT{g}", name=f"qT{g}") for g in range(G)]
        kT = [qkv.tile([P, S], BF16, tag=f"kT{g}", name=f"kT{g}") for g in range(G)]
        vt = qkv.tile([P, H, SB, 65], BF16, tag="vt")
        attp = top.enter_context(tc.tile_pool(name="attsb", bufs=5))

        # single PSUM pool for the whole kernel: tag "pa" = working (2 banks
        # x2), tag "po" = attention output accumulators (2 banks x2)
        ppool = top.enter_context(tc.tile_pool(name="ppool", bufs=2, space="PSUM"))

        attout = top.enter_context(tc.tile_pool(name="attout", bufs=1))
        rpool = top.enter_context(tc.tile_pool(name="rp", bufs=2))
        pstp = top.enter_context(tc.tile_pool(name="pstp", bufs=1))
        psts = {}
        lsbox = [None]

        # ---------------- attention (per stripe x head) + interleaved proj
        def emit_av(po, h, pend, lo, hi):
            sb, j = pend
            jlo = max(lo, j * P)
            for b in range(lo, hi, 512):
                clo, chi = max(jlo, b), b + 512
                if clo >= chi:
                    continue
                nc.tensor.matmul(po[:, clo - lo:chi - lo],
                                 lhsT=vt[:, h, j, :],
                                 rhs=sb[:, clo - lo:chi - lo],
                                 start=(j == 0), stop=(j == chi // P - 1))

        def emit_proj(si):
            tt, col = si * P // SW, (si * P) % SW
            pp = ppool.tile([P, E], F32, tag="pa", name="pp")
            for g in range(G):
                nc.tensor.matmul(pp, lhsT=attoutT[tt][:, g, col:col + P],
                                 rhs=wp_sb[:, g, :], start=(g == 0),
                                 stop=(g == G - 1))
            ob = out_pool.tile([P, E], F32, tag="ob", name="ob")
            nc.vector.tensor_add(out=ob, in0=pp, in1=bpb)
            nc.sync.dma_start(out=out_d[si * P:(si + 1) * P, :], in_=ob)

        def emit_att_head(t, h, prev_fin=None):
            """Emit one head's scores/exp/AV stream. The final two AV groups
            and the PSUM->SBUF staging are NOT emitted here — they are
            returned as a finisher closure that the NEXT head's emission
            calls after its first (independent) scores block, so the PE
            never head-of-line blocks on ScalarE's exp at head boundaries."""
            lo, hi = t * SW, (t + 1) * SW
            jmax = hi // P
            g, hh = h // 2, (h % 2) * D
            lsb = lsbox[0]
            po = ppool.tile([65, SW], F32, tag="po", name="po")
            pending = []
            for j in range(jmax):
                jlo = max(lo, j * P)
                ps = ppool.tile([P, SW], F32, tag="pa", name="ps")
                for b in range(lo, hi, 512):
                    clo, chi = max(jlo, b), b + 512
                    if clo >= chi:
                        continue
                    nc.tensor.matmul(ps[:, clo - lo:chi - lo],
                                     lhsT=kT[g][hh:hh + D, j * P:(j + 1) * P],
                                     rhs=qT[g][hh:hh + D, clo:chi],
                                     start=True, stop=True)
                if j * P >= lo:
                    # accumulate -1e30 strictly-lower mask onto diag block
                    nc.tensor.matmul(ps[:, j * P - lo:j * P - lo + P],
                                     lhsT=identb, rhs=umask,
                                     start=False, stop=True,
                                     skip_group_check=True)
                if j == 0 and prev_fin is not None:
                    prev_fin()
                if len(pending) >= 2:
                    emit_av(po, h, pending.pop(0), lo, hi)
                sb = attp.tile([P, SW], BF16, tag="attsb", name="sb")
                nc.scalar.activation(out=sb[:, jlo - lo:], in_=ps[:, jlo - lo:],
                                     func=mybir.ActivationFunctionType.Exp,
                                     scale=float(1.0 / np.sqrt(D)))
                pending.append((sb, j))

            def finish():
                for pend in pending:
                    emit_av(po, h, pend, lo, hi)
                # stage unnormalized output + denominator row to SBUF (frees
                # po); normalization happens batched per stripe
                pst = pstp.tile([D + 1, SW], F32, tag=f"pst{h}",
                                name=f"pst{h}")
                nc.vector.tensor_copy(out=pst, in_=po[0:D + 1, :])
                nc.sync.dma_start(out=lsb[h:h + 1, :], in_=pst[D:D + 1, :])
                psts[(t, h)] = pst

            return finish

        def emit_norm_stripe(t, lsb):
            # one DVE reciprocal for all 8 heads' denominators (the op costs
            # ~6.5 ns/free-elem regardless of partition count), then per
            # head: bf16 round (DVE), broadcast across 64 partitions via a
            # K=8 sel_h^T @ r matmul on the PE, multiply on DVE (PSUM read)
            rsb = rpool.tile([H, SW], F32, tag="rsb", name="rsb")
            nc.vector.reciprocal(out=rsb, in_=lsb)
            rsbb = rpool.tile([H, SW], BF16, tag="rsbb", name="rsbb")
            nc.vector.tensor_copy(out=rsbb, in_=rsb)
            for h in range(H):
                g, hh = h // 2, (h % 2) * D
                rrb = ppool.tile([D, SW], F32, tag="pa", name="rrb")
                for b in range(0, SW, 512):
                    nc.tensor.matmul(rrb[:, b:b + 512], lhsT=sel[h],
                                     rhs=rsbb[:, b:b + 512],
                                     start=True, stop=True)
                nc.vector.tensor_mul(out=attoutT[t][hh:hh + D, g, :],
                                     in0=psts[(t, h)][0:D, :], in1=rrb)

        with ExitStack() as ph2:
            xin = ph2.enter_context(tc.tile_pool(name="xin", bufs=8))
            xbp = ph2.enter_context(tc.tile_pool(name="xb", bufs=8))
            xt_pool = ph2.enter_context(tc.tile_pool(name="xt", bufs=1))
            wpool = ph2.enter_context(tc.tile_pool(name="wqkv", bufs=1))

            xT = [xt_pool.tile([P, S], BF16, tag=f"xT{e}", name=f"xT{e}")
                  for e in range(EB)]

            # ---------------- x load + bf16 cast + PE transpose -> xT
            # (emitted first so the x DMAs hit the queues before the weights)
            for k4 in range(SB // 4):
                xb = [xbp.tile([P, E], BF16, tag="xb", name="xb")
                      for _ in range(4)]
                for i in range(4):
                    si = k4 * 4 + i
                    xs = xin.tile([P, E], F32, tag="xs", name="xs")
                    nc.sync.dma_start(out=xs, in_=x_d[si * P:(si + 1) * P, :])
                    nc.vector.tensor_copy(out=xb[i], in_=xs)
                for ej in range(EB):
                    pt = ppool.tile([P, 512], BF16, tag="pa", name="pt")
                    for i in range(4):
                        nc.tensor.transpose(pt[:, i * P:(i + 1) * P],
                                            xb[i][:, ej * P:(ej + 1) * P],
                                            identb)
                    nc.vector.tensor_copy(out=xT[ej][:, k4 * 512:(k4 + 1) * 512],
                                          in_=pt)

            wq_r = wq_d.rearrange("h e d -> e h d")
            wk_r = wk_d.rearrange("h e d -> e h d")
            wv_r = wv_d.rearrange("h e d -> e h d")
            wq_sb = wpool.tile([P, EB, H, D], BF16, tag="wq")
            wk_sb = wpool.tile([P, EB, H, D], BF16, tag="wk")
            wv_sb = wpool.tile([P, EB, H, D], BF16, tag="wv")
            for w_r, w_sb, wnm in ((wq_r, wq_sb, "q"), (wk_r, wk_sb, "k"),
                                   (wv_r, wv_sb, "v")):
                wstg = wpool.tile([P, EB, H, D], F32, tag="wstg",
                                  name=f"wstg{wnm}", bufs=1)
                for ej in range(EB):
                    nc.sync.dma_start(out=wstg[:, ej],
                                      in_=w_r[ej * P:(ej + 1) * P, :, :])
                # cast on GpSimd: it idles until the first normalization,
                # while DVE's FIFO is busy ~18us with the x casts/copies —
                # this makes the weights ready as soon as their DMAs land
                nc.gpsimd.tensor_copy(out=w_sb, in_=wstg)
            wps = wpool.tile([P, G, E], F32, tag="wstg", name="wps", bufs=1)
            for g in range(G):
                nc.sync.dma_start(out=wps[:, g, :], in_=wp_d[g * P:(g + 1) * P, :])
            nc.gpsimd.tensor_copy(out=wp_sb, in_=wps)
            nc.sync.dma_start(
                out=bpb,
                in_=bass.AP(tensor=bp_d.tensor, offset=bp_d.offset,
                            ap=[[0, P]] + list(bp_d.ap)))
            nc.vector.memset(vt[:, :, :, 64:65], 1.0)

            # ---------------- v (4 heads per half so stripe-0 attention can
            # start as soon as the first half + its qkT group are done)
            def emit_v_half(p):
                h4 = 4 * p
                for si in range(SB):
                    pv = ppool.tile([P, E // 2], F32, tag="pa", name="pv")
                    for ej in range(EB):
                        nc.tensor.matmul(pv,
                                         lhsT=xT[ej][:, si * P:(si + 1) * P],
                                         rhs=wv_sb[:, ej, h4:h4 + 4, :],
                                         start=(ej == 0), stop=(ej == EB - 1))
                    nc.vector.tensor_copy(
                        out=vt[:, h4:h4 + 4, si, 0:64],
                        in_=pv.rearrange("p (h d) -> p h d", h=4))

            # ---------------- qT / kT (2 heads per group slice of wq_sb)
            def emit_qkt(g):
                for w_sb, dst in ((wq_sb, qT[g]), (wk_sb, kT[g])):
                    for q0 in range(0, S, QW):
                        pq = ppool.tile([P, QW], F32, tag="pa", name="pq")
                        for ej in range(EB):
                            for c in range(q0, q0 + QW, 512):
                                nc.tensor.matmul(
                                    pq[:, c - q0:c - q0 + 512],
                                    lhsT=w_sb[:, ej, 2 * g:2 * g + 2, :],
                                    rhs=xT[ej][:, c:c + 512],
                                    start=(ej == 0), stop=(ej == EB - 1))
                        nc.vector.tensor_copy(out=dst[:, q0:q0 + QW], in_=pq)

            attoutT = [attout.tile([P, G, SW], BF16, tag=f"attoutT{t}",
                                    name=f"attoutT{t}") for t in range(NS)]

            # interleave: qkt(g) [+ v half] + stripe-0 attention for its heads
            lsbs = {0: rpool.tile([H, SW], F32, tag="lsb", name="lsb0")}
            lsbox[0] = lsbs[0]
            fin = None
            for g in range(G):
                emit_qkt(g)
                if g < 2:
                    emit_v_half(g)
                fin = emit_att_head(0, 2 * g, fin)
                fin = emit_att_head(0, 2 * g + 1, fin)

        for t in range(1, NS):
            lsbs[t] = rpool.tile([H, SW], F32, tag="lsb", name=f"lsb{t}")
            lsbox[0] = lsbs[t]
            for h in range(H):
                fin = emit_att_head(t, h, fin)
                if h == 0:
                    emit_norm_stripe(t - 1, lsbs[t - 1])
                else:
                    emit_proj((t - 1) * SW // P + h - 1)
            emit_proj((t - 1) * SW // P + H - 1)
        fin()
        emit_norm_stripe(NS - 1, lsbs[NS - 1])
        for si in range((NS - 1) * SW // P, S // P):
            emit_proj(si)

    _hoist_matmul_waits(nc)
    return nc


def _hoist_matmul_waits(nc):
    """Several TRN2 ISA structs accept only one sync-wait slot; walrus dies
    with "Too many sync wait commands" otherwise. Hoist every wait of a
    multi-wait instruction onto same-engine NoOps inserted right before it
    (same engine queue => identical ordering semantics)."""
    nid = [0]
    for fn in nc.m.functions:
        for blk in fn.blocks:
            insts = blk.instructions
            out = []
            for inst in insts:
                si = inst.sync_info
                if (inst.engine != mybir.EngineType.Unassigned and si is not None
                        and len(si.on_wait) >= 2 and inst.opcode != "NoOp"):
                    for w in si.on_wait:
                        nid[0] += 1
                        nop = mybir.InstNoOp(name=f"I-mmwait-{nid[0]}",
                                             ins=[], outs=[])
                        nop.engine = inst.engine
                        nop.sync_info = mybir.SyncInfo(on_wait=[w], on_update=[])
                        nc.inst_map[nop.name] = nop
                        out.append(nop)
                    inst.sync_info = mybir.SyncInfo(on_wait=[],
                                                    on_update=list(si.on_update))
                out.append(inst)
            if len(out) != len(insts):
                insts[:] = out


_nc_cache = {}


def _get_nc(S=S_FULL):
    if S not in _nc_cache:
        _nc_cache[S] = build_attention_nc(S)
    return _nc_cache[S]


def kernel(x, Wq, Wk, Wv, Wp, bp, _trace=False):
    nc = _get_nc(x.shape[1])
    n = x.shape[0]
    wq = np.ascontiguousarray(Wq, np.float32)
    wk = np.ascontiguousarray(Wk, np.float32)
    wv = np.ascontiguousarray(Wv, np.float32)
    wp = np.ascontiguousarray(Wp, np.float32)
    bpc = np.ascontiguousarray(bp, np.float32)
    in_maps = [
        {"x": np.ascontiguousarray(x[b], np.float32),
         "Wq": wq, "Wk": wk, "Wv": wv, "Wp": wp, "bp": bpc}
        for b in range(n)
    ]
    res = bass_utils.run_bass_kernel_spmd(
        nc, in_maps, core_ids=list(range(n)), trace=_trace)
    out = np.stack([r["out"] for r in res.results], axis=0)
    if _trace:
        return out, res
    return out


# revision 69
# speedup vs baseline: 1.1719x; 1.1719x over previous
"""Multi-head causal attention (B=8, S=2048, E=512, H=8, D=64) on 8 trn2 cores.

Strategy: pure data parallelism over the batch dimension (B == n_cores == 8).
Each NeuronCore computes the full attention for one batch element; no
collectives are needed. All matmul operands are bf16 (stationary loads are
2 bytes/element and use standalone LDWEIGHTS instructions that the PE's
64-deep reorder window pulls ahead of in-flight MATMULs; the fp32r
self-loading path serializes ~270 ns of weight load into every matmul).
PSUM accumulation stays fp32. Per core:

  1. Load x [S,E], cast bf16 (DVE), transpose on PE -> xT [E,S] bf16.
  2. QKV projections:
       qT/kT: per 2-head group g, lhsT = Wq[e-block, (2h,d)=128] -> qT[g] [128,S]
       v:      per s-block, lhsT = xT block, rhs = Wv (4 heads at a time) ->
       v stored interleaved as vt[p=sk, h, j, 0:64] with vt[..., 64] = 1.0 so
       that the AV matmul's stationary operand [128, 65] also produces the
       softmax denominator (row 64 of the output accumulator). The v halves
       and the attention of stripe 0 are interleaved with the qT/kT groups so
       ScalarE's exp stream (the second-longest engine) starts early.
  3. Per sq-stripe (1024 cols) and head: scoresT[sk,sq] = kT_j^T @ qT
       (K=64), causal: only sq >= 128*j is computed; the diagonal 128x128
       block is masked by accumulating a bf16 (-1e30 strictly-lower) mask
       matmul into PSUM on the PE; exp via ScalarE (no max subtraction
       needed: |scores/8| <~ 2) -> bf16 att weights, then
       outT_aug[65,sq] += vt_j^T @ attT_j. Emission is software-pipelined two
       deep (scores_j, AV_{j-2}, exp_j) AND across heads: each head's final
       two AV groups + PSUM->SBUF staging are deferred into the next head's
       emission (after its first scores block), so the PE never head-of-line
       blocks on ScalarE's exp at head boundaries.
  4. Normalize, batched per stripe: each head's unnormalized output and
       denominator row l are staged to SBUF (l rows gathered onto 8
       partitions of one tile via SBUF->SBUF DMA); ONE DVE reciprocal per
       stripe covers all 8 heads (the op costs ~6.5 ns/free-elem regardless
       of partition count); r is rounded to bf16 and broadcast across 64
       partitions with a K=8 one-hot-row-selector matmul on the PE
       (sel_h^T @ r -> PSUM); attoutT[hd, s] = staged * r on DVE (PSUM
       read). The stripe-t norm chain is emitted after stripe t+1's first
       head so its serial DVE/DMA latency hides under PE work.
  5. Output projection out[s,e] = attoutT^T @ Wp + bp -> HBM, interleaved
       with the next stripe's attention.

A single PSUM pool with two tags (4+4 banks) is used for the whole kernel.
Post-scheduling, excess semaphore waits are hoisted onto same-engine NoOps
(several ISA structs accept only one wait slot; walrus rejects multi-wait
instructions)."""

import numpy as np
from contextlib import ExitStack

import concourse.bass as bass
import concourse.mybir as mybir
from concourse.tile import TileContext
from concourse.masks import make_identity
from concourse import bass_utils

F32 = mybir.dt.float32
BF16 = mybir.dt.bfloat16
B, S_FULL, E, H, D = 8, 2048, 512, 8, 64
P = 128
G = H // 2      # 2-head groups
EB = E // P     # e blocks
NEG = -1.0e30


def build_attention_nc(S=S_FULL):
    SB = S // P                 # s blocks
    SW = min(1024, S)           # stripe width (sq columns)
    NS = S // SW                # number of stripes
    QW = min(1024, S)           # qkT psum chunk width
    nc = bass.Bass(trn_type="TRN2")

    x_d = nc.dram_tensor("x", [S, E], F32, kind="ExternalInput").ap()
    wq_d = nc.dram_tensor("Wq", [H, E, D], F32, kind="ExternalInput").ap()
    wk_d = nc.dram_tensor("Wk", [H, E, D], F32, kind="ExternalInput").ap()
    wv_d = nc.dram_tensor("Wv", [H, E, D], F32, kind="ExternalInput").ap()
    wp_d = nc.dram_tensor("Wp", [E, E], F32, kind="ExternalInput").ap()
    bp_d = nc.dram_tensor("bp", [E], F32, kind="ExternalInput").ap()
    out_d = nc.dram_tensor("out", [S, E], F32, kind="ExternalOutput").ap()

    with TileContext(nc) as tc, ExitStack() as top:
        const = top.enter_context(tc.tile_pool(name="const", bufs=1))
        # bf16 identity: mask matmuls + bf16 transposes run at 1 cyc/row
        identb = const.tile([P, P], BF16, tag="identb")
        make_identity(nc, identb)
        # umask[sk, sq] = NEG where sq < sk else 0  (strictly-lower triangle)
        umask = const.tile([P, P], BF16, tag="umask")
        nc.gpsimd.memset(umask, 0.0)
        nc.gpsimd.affine_select(
            out=umask, in_=umask, compare_op=mybir.AluOpType.is_ge,
            fill=NEG, base=0, pattern=[[1, P]], channel_multiplier=-1,
        )
        bpb = const.tile([P, E], F32, tag="bpb")
        wp_sb = const.tile([P, G, E], BF16, tag="wp")
        # sel[h]: [H, D] one-hot row selector (row h all-ones) — K=8 matmul
        # lhsT that picks partition h of the rhs and broadcasts it 64-wide
        sel = []
        for h in range(H):
            s = const.tile([H, D], BF16, tag=f"sel{h}")
            nc.gpsimd.memset(s, 1.0)
            nc.gpsimd.affine_select(
                out=s, in_=s, compare_op=mybir.AluOpType.is_equal,
                fill=0.0, base=-h, pattern=[[0, D]], channel_multiplier=1,
            )
            sel.append(s)

        out_pool = top.enter_context(tc.tile_pool(name="outsb", bufs=2))
        qkv = top.enter_context(tc.tile_pool(name="qkv", bufs=1))
        qT = [qkv.tile([P, S], BF16, tag=f"qT{g}", name=f"qT{g}") for g in range(G)]
        kT = [qkv.tile([P, S], BF16, tag=f"kT{g}", name=f"kT{g}") for g in range(G)]
        vt = qkv.tile([P, H, SB, 65], BF16, tag="vt")
        attp = top.enter_context(tc.tile_pool(name="attsb", bufs=5))

        # single PSUM pool for the whole kernel: tag "pa" = working (2 banks
        # x2), tag "po" = attention output accumulators (2 banks x2)
        ppool = top.enter_context(tc.tile_pool(name="ppool", bufs=2, space="PSUM"))

        attout = top.enter_context(tc.tile_pool(name="attout", bufs=1))
        rpool = top.enter_context(tc.tile_pool(name="rp", bufs=2))
        pstp = top.enter_context(tc.tile_pool(name="pstp", bufs=1))
        psts = {}
        lsbox = [None]

        # ---------------- attention (per stripe x head) + interleaved proj
        def emit_av(po, h, pend, lo, hi):
            sb, j = pend
            jlo = max(lo, j * P)
            for b in range(lo, hi, 512):
                clo, chi = max(jlo, b), b + 512
                if clo >= chi:
                    continue
                nc.tensor.matmul(po[:, clo - lo:chi - lo],
                                 lhsT=vt[:, h, j, :],
                                 rhs=sb[:, clo - lo:chi - lo],
                                 start=(j == 0), stop=(j == chi // P - 1))

        def emit_proj(si):
            tt, col = si * P // SW, (si * P) % SW
            pp = ppool.tile([P, E], F32, tag="pa", name="pp")
            for g in range(G):
                nc.tensor.matmul(pp, lhsT=attoutT[tt][:, g, col:col + P],
                                 rhs=wp_sb[:, g, :], start=(g == 0),
                                 stop=(g == G - 1))
            ob = out_pool.tile([P, E], F32, tag="ob", name="ob")
            nc.vector.tensor_add(out=ob, in0=pp, in1=bpb)
            nc.sync.dma_start(out=out_d[si * P:(si + 1) * P, :], in_=ob)

        def emit_att_head(t, h, prev_fin=None):
            """Emit one head's scores/exp/AV stream. The final two AV groups
            and the PSUM->SBUF staging are NOT emitted here — they are
            returned as a finisher closure that the NEXT head's emission
            calls after its first (independent) scores block, so the PE
            never head-of-line blocks on ScalarE's exp at head boundaries."""
            lo, hi = t * SW, (t + 1) * SW
            jmax = hi // P
            g, hh = h // 2, (h % 2) * D
            lsb = lsbox[0]
            po = ppool.tile([65, SW], F32, tag="po", name="po")
            pending = []
            for j in range(jmax):
                jlo = max(lo, j * P)
                ps = ppool.tile([P, SW], F32, tag="pa", name="ps")
                for b in range(lo, hi, 512):
                    clo, chi = max(jlo, b), b + 512
                    if clo >= chi:
                        continue
                    nc.tensor.matmul(ps[:, clo - lo:chi - lo],
                                     lhsT=kT[g][hh:hh + D, j * P:(j + 1) * P],
                                     rhs=qT[g][hh:hh + D, clo:chi],
                                     start=True, stop=True)
                if j * P >= lo:
                    # accumulate -1e30 strictly-lower mask onto diag block
                    nc.tensor.matmul(ps[:, j * P - lo:j * P - lo + P],
                                     lhsT=identb, rhs=umask,
                                     start=False, stop=True,
                                     skip_group_check=True)
                if j == 0 and prev_fin is not None:
                    prev_fin()
                if len(pending) >= 2:
                    emit_av(po, h, pending.pop(0), lo, hi)
                sb = attp.tile([P, SW], BF16, tag="attsb", name="sb")
                nc.scalar.activation(out=sb[:, jlo - lo:], in_=ps[:, jlo - lo:],
                                     func=mybir.ActivationFunctionType.Exp,
                                     scale=float(1.0 / np.sqrt(D)))
                pending.append((sb, j))

            def finish():
                for pend in pending:
                    emit_av(po, h, pend, lo, hi)
                # stage unnormalized output + denominator row to SBUF (frees
                # po); normalization happens batched per stripe
                pst = pstp.tile([D + 1, SW], F32, tag=f"pst{h}",
                                name=f"pst{h}")
                nc.vector.tensor_copy(out=pst, in_=po[0:D + 1, :])
                nc.sync.dma_start(out=lsb[h:h + 1, :], in_=pst[D:D + 1, :])
                psts[(t, h)] = pst

            return finish

        def emit_norm_stripe(t, lsb):
            # one DVE reciprocal for all 8 heads' denominators (the op costs
            # ~6.5 ns/free-elem regardless of partition count), then per
            # head: bf16 round (DVE), broadcast across 64 partitions via a
            # K=8 sel_h^T @ r matmul on the PE, multiply on DVE (PSUM read)
            rsb = rpool.tile([H, SW], F32, tag="rsb", name="rsb")
            nc.vector.reciprocal(out=rsb, in_=lsb)
            rsbb = rpool.tile([H, SW], BF16, tag="rsbb", name="rsbb")
            nc.vector.tensor_copy(out=rsbb, in_=rsb)
            for h in range(H):
                g, hh = h // 2, (h % 2) * D
                rrb = ppool.tile([D, SW], F32, tag="pa", name="rrb")
                for b in range(0, SW, 512):
                    nc.tensor.matmul(rrb[:, b:b + 512], lhsT=sel[h],
                                     rhs=rsbb[:, b:b + 512],
                                     start=True, stop=True)
                nc.vector.tensor_mul(out=attoutT[t][hh:hh + D, g, :],
                                     in0=psts[(t, h)][0:D, :], in1=rrb)

        with ExitStack() as ph2:
            xin = ph2.enter_context(tc.tile_pool(name="xin", bufs=8))
            xbp = ph2.enter_context(tc.tile_pool(name="xb", bufs=8))
            xt_pool = ph2.enter_context(tc.tile_pool(name="xt", bufs=1))
            wpool = ph2.enter_context(tc.tile_pool(name="wqkv", bufs=1))

            xT = [xt_pool.tile([P, S], BF16, tag=f"xT{e}", name=f"xT{e}")
                  for e in range(EB)]

            # ---------------- x load + bf16 cast + PE transpose -> xT
            # (emitted first so the x DMAs hit the queues before the weights)
            for k4 in range(SB // 4):
                xb = [xbp.tile([P, E], BF16, tag="xb", name="xb")
                      for _ in range(4)]
                for i in range(4):
                    si = k4 * 4 + i
                    xs = xin.tile([P, E], F32, tag="xs", name="xs")
                    nc.sync.dma_start(out=xs, in_=x_d[si * P:(si + 1) * P, :])
                    nc.vector.tensor_copy(out=xb[i], in_=xs)
                for ej in range(EB):
                    pt = ppool.tile([P, 512], BF16, tag="pa", name="pt")
                    for i in range(4):
                        nc.tensor.transpose(pt[:, i * P:(i + 1) * P],
                                            xb[i][:, ej * P:(ej + 1) * P],
                                            identb)
                    nc.vector.tensor_copy(out=xT[ej][:, k4 * 512:(k4 + 1) * 512],
                                          in_=pt)

            wq_r = wq_d.rearrange("h e d -> e h d")
            wk_r = wk_d.rearrange("h e d -> e h d")
            wv_r = wv_d.rearrange("h e d -> e h d")
            wq_sb = wpool.tile([P, EB, H, D], BF16, tag="wq")
            wk_sb = wpool.tile([P, EB, H, D], BF16, tag="wk")
            wv_sb = wpool.tile([P, EB, H, D], BF16, tag="wv")
            for w_r, w_sb, wnm in ((wq_r, wq_sb, "q"), (wk_r, wk_sb, "k"),
                                   (wv_r, wv_sb, "v")):
                wstg = wpool.tile([P, EB, H, D], F32, tag="wstg",
                                  name=f"wstg{wnm}", bufs=1)
                for ej in range(EB):
                    nc.sync.dma_start(out=wstg[:, ej],
                                      in_=w_r[ej * P:(ej + 1) * P, :, :])
                nc.vector.tensor_copy(out=w_sb, in_=wstg)
            wps = wpool.tile([P, G, E], F32, tag="wstg", name="wps", bufs=1)
            for g in range(G):
                nc.sync.dma_start(out=wps[:, g, :], in_=wp_d[g * P:(g + 1) * P, :])
            nc.vector.tensor_copy(out=wp_sb, in_=wps)
            nc.sync.dma_start(
                out=bpb,
                in_=bass.AP(tensor=bp_d.tensor, offset=bp_d.offset,
                            ap=[[0, P]] + list(bp_d.ap)))
            nc.vector.memset(vt[:, :, :, 64:65], 1.0)

            # ---------------- v (4 heads per half so stripe-0 attention can
            # start as soon as the first half + its qkT group are done)
            def emit_v_half(p):
                h4 = 4 * p
                for si in range(SB):
                    pv = ppool.tile([P, E // 2], F32, tag="pa", name="pv")
                    for ej in range(EB):
                        nc.tensor.matmul(pv,
                                         lhsT=xT[ej][:, si * P:(si + 1) * P],
                                         rhs=wv_sb[:, ej, h4:h4 + 4, :],
                                         start=(ej == 0), stop=(ej == EB - 1))
                    nc.vector.tensor_copy(
                        out=vt[:, h4:h4 + 4, si, 0:64],
                        in_=pv.rearrange("p (h d) -> p h d", h=4))

            # ---------------- qT / kT (2 heads per group slice of wq_sb)
            def emit_qkt(g):
                for w_sb, dst in ((wq_sb, qT[g]), (wk_sb, kT[g])):
                    for q0 in range(0, S, QW):
                        pq = ppool.tile([P, QW], F32, tag="pa", name="pq")
                        for ej in range(EB):
                            for c in range(q0, q0 + QW, 512):
                                nc.tensor.matmul(
                                    pq[:, c - q0:c - q0 + 512],
                                    lhsT=w_sb[:, ej, 2 * g:2 * g + 2, :],
                                    rhs=xT[ej][:, c:c + 512],
                                    start=(ej == 0), stop=(ej == EB - 1))
                        nc.vector.tensor_copy(out=dst[:, q0:q0 + QW], in_=pq)

            attoutT = [attout.tile([P, G, SW], BF16, tag=f"attoutT{t}",
                                    name=f"attoutT{t}") for t in range(NS)]

            # interleave: qkt(g) [+ v half] + stripe-0 attention for its heads
            lsbs = {0: rpool.tile([H, SW], F32, tag="lsb", name="lsb0")}
            lsbox[0] = lsbs[0]
            fin = None
            for g in range(G):
                emit_qkt(g)
                if g < 2:
                    emit_v_half(g)
                fin = emit_att_head(0, 2 * g, fin)
                fin = emit_att_head(0, 2 * g + 1, fin)

        for t in range(1, NS):
            lsbs[t] = rpool.tile([H, SW], F32, tag="lsb", name=f"lsb{t}")
            lsbox[0] = lsbs[t]
            for h in range(H):
                fin = emit_att_head(t, h, fin)
                if h == 0:
                    emit_norm_stripe(t - 1, lsbs[t - 1])
                else:
                    emit_proj((t - 1) * SW // P + h - 1)
            emit_proj((t - 1) * SW // P + H - 1)
        fin()
        emit_norm_stripe(NS - 1, lsbs[NS - 1])
        for si in range((NS - 1) * SW // P, S // P):
            emit_proj(si)

    _hoist_matmul_waits(nc)
    return nc


def _hoist_matmul_waits(nc):
    """Several TRN2 ISA structs accept only one sync-wait slot; walrus dies
    with "Too many sync wait commands" otherwise. Hoist every wait of a
    multi-wait instruction onto same-engine NoOps inserted right before it
    (same engine queue => identical ordering semantics)."""
    nid = [0]
    for fn in nc.m.functions:
        for blk in fn.blocks:
            insts = blk.instructions
            out = []
            for inst in insts:
                si = inst.sync_info
                if (inst.engine != mybir.EngineType.Unassigned and si is not None
                        and len(si.on_wait) >= 2 and inst.opcode != "NoOp"):
                    for w in si.on_wait:
                        nid[0] += 1
                        nop = mybir.InstNoOp(name=f"I-mmwait-{nid[0]}",
                                             ins=[], outs=[])
                        nop.engine = inst.engine
                        nop.sync_info = mybir.SyncInfo(on_wait=[w], on_update=[])
                        nc.inst_map[nop.name] = nop
                        out.append(nop)
                    inst.sync_info = mybir.SyncInfo(on_wait=[],
                                                    on_update=list(si.on_update))
                out.append(inst)
            if len(out) != len(insts):
                insts[:] = out


_nc_cache = {}


def _get_nc(S=S_FULL):
    if S not in _nc_cache:
        _nc_cache[S] = build_attention_nc(S)
    return _nc_cache[S]


def kernel(x, Wq, Wk, Wv, Wp, bp, _trace=False):
    nc = _get_nc(x.shape[1])
    n = x.shape[0]
    wq = np.ascontiguousarray(Wq, np.float32)
    wk = np.ascontiguousarray(Wk, np.float32)
    wv = np.ascontiguousarray(Wv, np.float32)
    wp = np.ascontiguousarray(Wp, np.float32)
    bpc = np.ascontiguousarray(bp, np.float32)
    in_maps = [
        {"x": np.ascontiguousarray(x[b], np.float32),
         "Wq": wq, "Wk": wk, "Wv": wv, "Wp": wp, "bp": bpc}
        for b in range(n)
    ]
    res = bass_utils.run_bass_kernel_spmd(
        nc, in_maps, core_ids=list(range(n)), trace=_trace)
    out = np.stack([r["out"] for r in res.results], axis=0)
    if _trace:
        return out, res
    return out
